# revision 11
# baseline (speedup 1.0000x reference)
"""CGC (Customized Gate Control) MoE layer on 8 Trainium2 NeuronCores.

Strategy: data-parallel over batch (B=4096 -> 8 shards of 512 rows); every
core computes all 8 expert MLPs for its shard — no collectives.

Precision/speed scheme (~1.7e-3 rel err vs the f32 reference on hw):
  - x, W1, Wg are uploaded as natural-scale fp8e4m3 (hi, res) pairs:
    a ~ hi + res with res = q8(a - hi), giving ~0.15% representation error.
  - Layer-1 / gate matmuls run as fp8 DoubleRow (2 contraction rows per
    instruction at 0.5 cycles/row): the three significant cross products
    (hi*hi, hi*res, res*hi) are computed by three DR instructions per
    k-tile pair using strided (hi,res) slices — 0.75 c/row/k-tile vs
    float32r's 1.0, with no operand duplication.
  - h, W2, b2, expert outputs and y are fp16 (L2 matmul at 1.0 c/row).
  - Per-tensor pow2 scales are chosen on the host; descale factors ride in
    as a small aux tensor and are applied via the ScalarE activation's
    per-partition scale operand, so nothing is baked into the compiled NEFF.

Per-core dataflow:
  - x arrives pre-transposed from the host as xp [128, kt, (hi,res), B] —
    no PE transposes or PSUM round-trips for inputs.
  - L1: hT[h1, b] psum group of 12 DR matmuls per m-tile; ScalarE applies
    relu + per-partition b1 bias + descale, writing fp16.
  - L2: oe[b, H2] fp16 matmuls; bias-add (host-precomputed b2 broadcast
    tile) + relu on VectorE.
  - Gates: DR logits (stationary padded to 16 cols for the DR ldweights
    stride rule), ScalarE descale+bias, PE transpose, softmax.
  - Gated combine: single-instruction MACs (scalar_tensor_tensor) on
    VectorE, interleaved per batch-tile into the L2 pipeline.
  - Output DMAs and the b2 broadcast load run on the idle Pool engine's
    DGE queue so they never head-of-line block the SP weight-load queue.
"""

import numpy as np
import ml_dtypes

import concourse.tile as tile
from concourse import bacc, mybir
from concourse.bass_utils import run_bass_kernel_spmd

N_CORES = 8
B = 4096
BL = B // N_CORES  # 512 rows per core
D = 1024
H1 = 1024
H2 = 512
DOM = 3
NES = 2
NSH = 2
E_SPEC = DOM * NES  # 6
GATE_K = NES + NSH  # 4
TOTAL_E = E_SPEC + NSH  # 8

F8 = mybir.dt.float8e4
F16 = mybir.dt.float16
F32 = mybir.dt.float32
AX = mybir.AxisListType
AF = mybir.ActivationFunctionType
ALU = mybir.AluOpType
DR = mybir.MatmulPerfMode.DoubleRow

NBT = BL // 128  # 4 batch tiles per core
NKD = D // 128   # 8 contraction tiles over D
NG = NKD // 2    # 4 DoubleRow k-tile pairs
NKH = H1 // 128  # 8 contraction tiles over H1
NMH = H1 // 128  # 8 output tiles over H1

NP8 = ml_dtypes.float8_e4m3fn
KPAD = 16  # gate stationary column padding (DR ldweights stride rule)

# aux tensor column map (f32 [128, 16]):
#   0..7   : L1 descale per expert e (broadcast down partitions)
#   8..11  : gate descale per gate g=0..3 (3=shared)
#   12..14 : bg[d] in rows 0..3
#   15     : bsg in rows 0..7
AUX_DSC1 = 0
AUX_DSCG = 8
AUX_BG = 12
AUX_BSG = 15


def _build_nc():
    from contextlib import ExitStack

    nc = bacc.Bacc("TRN2", target_bir_lowering=False, debug=False)

    xps = [
        nc.dram_tensor(f"xp{i}", [128, NKD, 2, BL], F8, kind="ExternalInput")
        for i in range(4)
    ]
    W1p = nc.dram_tensor("W1p", [TOTAL_E, 128, NMH, NKD, 2, 128], F8,
                         kind="ExternalInput")
    W2p = nc.dram_tensor("W2p", [TOTAL_E, 128, NKH, H2], F16, kind="ExternalInput")
    b1p = nc.dram_tensor("b1p", [128, TOTAL_E, NMH], F32, kind="ExternalInput")
    b2bc = nc.dram_tensor("b2bc", [128, TOTAL_E, H2], F16, kind="ExternalInput")
    Wgp = nc.dram_tensor("Wgp", [DOM, 128, NKD, 2, KPAD], F8, kind="ExternalInput")
    Wsgp = nc.dram_tensor("Wsgp", [128, NKD, 2, KPAD], F8, kind="ExternalInput")
    aux = nc.dram_tensor("aux", [128, 16], F32, kind="ExternalInput")
    ys = [
        nc.dram_tensor(n, [BL, H2], F16, kind="ExternalOutput")
        for n in ("y0", "y1", "y2", "ysh")
    ]

    with tile.TileContext(nc) as tc, ExitStack() as ctx:
        p_const = ctx.enter_context(tc.tile_pool(name="const", bufs=1))
        p_xp = ctx.enter_context(tc.tile_pool(name="xp", bufs=2))
        p_w1 = ctx.enter_context(tc.tile_pool(name="w1", bufs=4))
        p_w2 = ctx.enter_context(tc.tile_pool(name="w2", bufs=2))
        p_h = ctx.enter_context(tc.tile_pool(name="hT", bufs=2))
        p_oe = ctx.enter_context(tc.tile_pool(name="oe", bufs=2))
        p_osh = ctx.enter_context(tc.tile_pool(name="osh", bufs=1))
        p_acc = ctx.enter_context(tc.tile_pool(name="acc", bufs=1))
        p_gw = ctx.enter_context(tc.tile_pool(name="gw", bufs=1))
        p_gt = ctx.enter_context(tc.tile_pool(name="gt", bufs=2))
        p_sm = ctx.enter_context(tc.tile_pool(name="sm", bufs=3))
        p_tmp = ctx.enter_context(tc.tile_pool(name="tmp", bufs=2))
        ps_h = ctx.enter_context(tc.tile_pool(name="psh", bufs=2, space="PSUM"))
        ps_o = ctx.enter_context(tc.tile_pool(name="pso", bufs=2, space="PSUM"))
        ps_t = ctx.enter_context(tc.tile_pool(name="pst", bufs=2, space="PSUM"))

        # aux (descales + gate biases) first: gate ACT depends on it.
        aux_sb = p_const.tile([128, 16], F32)
        nc.sync.dma_start(out=aux_sb, in_=aux[:])
        # Identity for gate transposes, built on-chip.
        ident_sb = p_const.tile([128, 128], F32)
        nc.gpsimd.memset(ident_sb, 0.0)
        nc.gpsimd.affine_select(
            out=ident_sb,
            in_=ident_sb,
            compare_op=ALU.not_equal,
            fill=1.0,
            base=0,
            pattern=[[-1, 128]],
            channel_multiplier=1,
        )
        # b2 broadcast tiles ride the idle Pool DGE queue.
        b2_sb = p_const.tile([128, TOTAL_E, H2], F16)
        nc.gpsimd.dma_start(out=b2_sb, in_=b2bc[:])
        # PE warm-up while the first DMAs are in flight (p-state ramp).
        for _ in range(16):
            pw = ps_t.tile([128, 128], F32, tag="pt", name="pw")
            nc.tensor.matmul(pw, lhsT=ident_sb, rhs=ident_sb, start=True, stop=True)

        def load_xp(i):
            xp = p_xp.tile([128, NKD, 2, BL], F8, tag="xp")
            nc.sync.dma_start(out=xp, in_=xps[i][:])
            return xp

        def mm3(pg, w_sb, xp, g, start, stop):
            """Three-term DR matmuls for k-tile pair g into psum pg."""
            sl = slice(2 * g, 2 * g + 2)
            nc.tensor.matmul(pg, lhsT=w_sb[:, sl, 0, :], rhs=xp[:, sl, 0, :],
                             start=start, stop=False, perf_mode=DR)
            nc.tensor.matmul(pg, lhsT=w_sb[:, sl, 0, :], rhs=xp[:, sl, 1, :],
                             start=False, stop=False, perf_mode=DR)
            nc.tensor.matmul(pg, lhsT=w_sb[:, sl, 1, :], rhs=xp[:, sl, 0, :],
                             start=False, stop=stop, perf_mode=DR)

        def compute_gate(xp, wg_dram, gi, K, tag):
            """softmax(x @ Wg + bg) -> gw tile [128, NBT, K] (b on partitions)."""
            wg_sb = p_sm.tile([128, NKD, 2, KPAD], F8, tag="wg")
            nc.sync.dma_start(out=wg_sb, in_=wg_dram)
            pg = ps_t.tile([KPAD, BL], F32, tag="pt")
            for g in range(NG):
                mm3(pg, wg_sb, xp, g, start=(g == 0), stop=(g == NG - 1))
            glT = p_gt.tile([K, BL], F32, tag="glT")
            if gi < DOM:
                bias_ap = aux_sb[:K, AUX_BG + gi : AUX_BG + gi + 1]
            else:
                bias_ap = aux_sb[:K, AUX_BSG : AUX_BSG + 1]
            nc.scalar.activation(
                out=glT, in_=pg[:K, :], func=AF.Identity, bias=bias_ap,
                scale=aux_sb[:K, AUX_DSCG + gi : AUX_DSCG + gi + 1],
            )
            gw = p_gw.tile([128, NBT, K], F32, tag=tag)
            for bt in range(NBT):
                ptg = ps_t.tile([128, K], F32, tag="pt")
                nc.tensor.transpose(
                    ptg, glT[:, bt * 128 : (bt + 1) * 128], ident_sb[:K, :K]
                )
                nm = p_sm.tile([128, 1], F32, tag="nm")
                nc.vector.reduce_max(out=nm, in_=ptg, axis=AX.X, negate=True)
                esb = p_sm.tile([128, K], F32, tag="esb")
                nc.scalar.activation(
                    out=esb, in_=ptg, func=AF.Exp, bias=nm, scale=1.0
                )
                ssb = p_sm.tile([128, 1], F32, tag="ssb")
                nc.vector.reduce_sum(out=ssb, in_=esb, axis=AX.X)
                rsb = p_sm.tile([128, 1], F32, tag="rsb")
                nc.vector.reciprocal(out=rsb, in_=ssb)
                nc.vector.tensor_scalar_mul(gw[:, bt, :], esb, rsb)
            return gw

        def expert(xp, e, out_pool, tag, macs=()):
            """Two-layer MLP -> oe [128, NBT, H2] fp16.

            macs: per-bt gated-combine hooks (acc_idx, gw, col, first),
            emitted right after each batch-tile's relu so VectorE work
            pipelines with the next tile's matmuls.
            """
            hT = p_h.tile([128, NMH, BL], F16, tag="hT")
            for half in range(2):
                w1_sb = p_w1.tile([128, NMH // 2, NKD, 2, 128], F8, tag="w1")
                nc.sync.dma_start(
                    out=w1_sb,
                    in_=W1p[e][:, half * (NMH // 2) : (half + 1) * (NMH // 2)],
                )
                for mi in range(NMH // 2):
                    mt = half * (NMH // 2) + mi
                    ph = ps_h.tile([128, BL], F32, tag="ph")
                    for g in range(NG):
                        mm3(ph, w1_sb[:, mi], xp, g,
                            start=(g == 0), stop=(g == NG - 1))
                    nc.scalar.activation(
                        out=hT[:, mt, :],
                        in_=ph,
                        func=AF.Relu,
                        bias=b1_sb[:, e, mt : mt + 1],
                        scale=aux_sb[:, AUX_DSC1 + e : AUX_DSC1 + e + 1],
                    )
            w2_sb = p_w2.tile([128, NKH, H2], F16, tag="w2")
            nc.sync.dma_start(out=w2_sb, in_=W2p[e])
            oe = out_pool.tile([128, NBT, H2], F16, tag=tag)
            for bt in range(NBT):
                po = ps_o.tile([128, H2], F32, tag="po")
                for kt in range(NKH):
                    nc.tensor.matmul(
                        po,
                        lhsT=hT[:, kt, bt * 128 : (bt + 1) * 128],
                        rhs=w2_sb[:, kt, :],
                        start=(kt == 0),
                        stop=(kt == NKH - 1),
                    )
                z = p_tmp.tile([128, H2], F16, tag="z")
                nc.vector.tensor_tensor(z, po, b2_sb[:, e, :], ALU.add)
                nc.vector.tensor_scalar_max(oe[:, bt, :], z, 0.0)
                for acc_idx, gw, col, first in macs:
                    mac(acc_idx, oe, gw, col, bt, first)
            return oe

        accs = [None] * 4

        def mac(acc_idx, oe, gw, col, bt, first):
            acc = accs[acc_idx]
            sc = gw[:, bt, col : col + 1]
            if first:
                nc.vector.tensor_scalar_mul(acc[:, bt, :], oe[:, bt, :], sc)
            else:
                nc.vector.scalar_tensor_tensor(
                    out=acc[:, bt, :],
                    in0=oe[:, bt, :],
                    scalar=sc,
                    in1=acc[:, bt, :],
                    op0=ALU.mult,
                    op1=ALU.add,
                )

        def accumulate(acc_idx, oe, gw, col, first):
            for bt in range(NBT):
                mac(acc_idx, oe, gw, col, bt, first)

        # ---- shared phase: shared experts kept resident, shared gate ----
        xp_sh = load_xp(3)
        gws = compute_gate(xp_sh, Wsgp[:], DOM, TOTAL_E, tag="gws")
        b1_sb = p_const.tile([128, TOTAL_E, NMH], F32)
        nc.sync.dma_start(out=b1_sb, in_=b1p[:])
        accs[3] = p_acc.tile([128, NBT, H2], F16, tag="acc3", name="acc3")
        osh = []
        xp_next = None
        for j in range(NSH):
            o = expert(
                xp_sh, E_SPEC + j, p_osh, tag=f"osh{j}",
                macs=[(3, gws, E_SPEC + j, j == 0)],
            )
            osh.append(o)
            if j == 0:
                xp_next = load_xp(0)

        # ---- domain phases ----
        for d in range(DOM):
            xp_d = xp_next
            gw_d = compute_gate(xp_d, Wgp[d], d, GATE_K, tag=f"gw{d}")
            accs[d] = p_acc.tile(
                [128, NBT, H2], F16, tag=f"acc{d}", name=f"acc{d}"
            )
            accumulate(d, osh[0], gw_d, NES + 0, first=True)
            accumulate(d, osh[1], gw_d, NES + 1, first=False)
            for i in range(NES):
                e = d * NES + i
                oe = expert(
                    xp_d, e, p_oe, tag="oe",
                    macs=[(d, gw_d, i, False), (3, gws, e, False)],
                )
                if i == 0 and d < DOM - 1:
                    xp_next = load_xp(d + 1)
            nc.gpsimd.dma_start(
                out=ys[d][:].rearrange("(bt p) o -> p bt o", p=128), in_=accs[d]
            )
        nc.gpsimd.dma_start(
            out=ys[3][:].rearrange("(bt p) o -> p bt o", p=128), in_=accs[3]
        )

    nc.compile()
    return nc


_NC_CACHE = {}


def _get_nc():
    if "nc" not in _NC_CACHE:
        _NC_CACHE["nc"] = _build_nc()
    return _NC_CACHE["nc"]


def _pow2_scale(a, target=192.0):
    m = float(np.abs(a).max())
    if m == 0.0 or not np.isfinite(m):
        return 1.0
    return float(2.0 ** np.floor(np.log2(target / m)))


def _q8(a):
    return a.astype(NP8)


def _pair(a, target=192.0):
    """a*s ~ hi + res (both natural-scale fp8). Returns (hi, res, s)."""
    s = _pow2_scale(a, target)
    asc = (a * s).astype(np.float32)
    hi = _q8(asc)
    res = _q8(asc - hi.astype(np.float32))
    return hi, res, s


def _pack_xT(x, s):
    """[BL, D] f32 -> [128, NKD, 2, BL] fp8 pair layout (d on partitions)."""
    asc = (x.astype(np.float32) * s)
    hi = _q8(asc)
    res = _q8(asc - hi.astype(np.float32))
    out = np.empty((128, NKD, 2, x.shape[0]), dtype=NP8)
    for t, arr in enumerate((hi, res)):
        # arr [BL, D] -> T [D, BL] -> [NKD, 128, BL] -> [128, NKD, BL]
        out[:, :, t, :] = arr.T.reshape(NKD, 128, -1).transpose(1, 0, 2)
    return out


def _pack_w1(Wall):
    """[E, D, H1] f32 -> ([E, 128, NMH, NKD, 2, 128] fp8, scales[E])."""
    out = np.empty((TOTAL_E, 128, NMH, NKD, 2, 128), dtype=NP8)
    scales = np.empty(TOTAL_E, dtype=np.float64)
    for e in range(TOTAL_E):
        hi, res, s = _pair(Wall[e])
        scales[e] = s
        for t, arr in enumerate((hi, res)):
            # arr [D, H1] -> [NKD, 128p, NMH, 128m] -> [128p, NMH, NKD, 128m]
            out[e, :, :, :, t, :] = (
                arr.reshape(NKD, 128, NMH, 128).transpose(1, 2, 0, 3)
            )
    return out, scales


def _pack_wg(Wg, K):
    """[D, K] f32 -> ([128, NKD, 2, KPAD] fp8 zero-padded, scale)."""
    hi, res, s = _pair(Wg)
    out = np.zeros((128, NKD, 2, KPAD), dtype=NP8)
    for t, arr in enumerate((hi, res)):
        out[:, :, t, :K] = arr.reshape(NKD, 128, K).transpose(1, 0, 2)
    return out, s


def kernel(**inputs):
    return run_kernel(inputs)


def run_kernel(inputs, trace=False):
    nc = _get_nc()
    f = {k: np.ascontiguousarray(np.asarray(v, dtype=np.float32))
         for k, v in inputs.items()}

    W1all = np.concatenate([f["W1s"], f["W1h"]], axis=0)
    W2all = np.concatenate([f["W2s"], f["W2h"]], axis=0)
    b1all = np.concatenate([f["b1s"], f["b1h"]], axis=0)
    b2all = np.concatenate([f["b2s"], f["b2h"]], axis=0)

    W1p, s1 = _pack_w1(W1all)
    W2p = W2all.reshape(TOTAL_E, NKH, 128, H2).transpose(0, 2, 1, 3).astype(np.float16)
    W2p = np.ascontiguousarray(W2p)
    # b1p[p, e, mt] = b1[e, mt*128 + p]
    b1p = np.ascontiguousarray(b1all.reshape(TOTAL_E, NMH, 128).transpose(2, 0, 1))
    b2bc = np.ascontiguousarray(
        np.broadcast_to(b2all.astype(np.float16)[None], (128, TOTAL_E, H2))
    )

    xs_full = [f["x0"], f["x1"], f["x2"], f["x_shared"]]
    sx = [_pow2_scale(x) for x in xs_full]

    wg_packs = [_pack_wg(f["Wg"][d], GATE_K) for d in range(DOM)]
    Wgp = np.ascontiguousarray(np.stack([w for w, _ in wg_packs]))
    Wsgp, sgs = _pack_wg(f["Wsg"], TOTAL_E)
    Wsgp = np.ascontiguousarray(Wsgp)

    aux = np.zeros((128, 16), dtype=np.float32)
    for e in range(TOTAL_E):
        xd = e // NES if e < E_SPEC else 3
        aux[:, AUX_DSC1 + e] = 1.0 / (sx[xd] * s1[e])
    for d in range(DOM):
        aux[:, AUX_DSCG + d] = 1.0 / (sx[d] * wg_packs[d][1])
        aux[:GATE_K, AUX_BG + d] = f["bg"][d]
    aux[:, AUX_DSCG + 3] = 1.0 / (sx[3] * sgs)
    aux[:TOTAL_E, AUX_BSG] = f["bsg"]

    common = {
        "W1p": W1p, "W2p": W2p, "b1p": b1p, "b2bc": b2bc,
        "Wgp": Wgp, "Wsgp": Wsgp, "aux": aux,
    }
    in_maps = []
    for c in range(N_CORES):
        m = dict(common)
        for i, name in enumerate(("x0", "x1", "x2", "x_shared")):
            shard = f[name][c * BL : (c + 1) * BL]
            m[f"xp{i}"] = _pack_xT(shard, sx[i])
        in_maps.append(m)

    res = run_bass_kernel_spmd(nc, in_maps, list(range(N_CORES)), trace=trace)
    outs = []
    for name in ("y0", "y1", "y2", "ysh"):
        outs.append(
            np.concatenate(
                [np.asarray(res.results[c][name]).astype(np.float32)
                 for c in range(N_CORES)],
                axis=0,
            )
        )
    out = tuple(outs)
    if trace:
        return out, res
    return out
